# revision 9
# baseline (speedup 1.0000x reference)
"""Trainium2 Bass kernel for nn_MaskedPosmap2Normal.

Per batch image b and pixel (i,j), the reference computes
    d_k = neighbor_k - center  (k = right, up, left, down; zero-padded)
    normal = sum_k valid_k * (d_k x d_{k+1 mod 4})
    out = normal / max(||normal||, 1e-12)
where valid_k is the AND of the 3 mask bits bracketing directions k, k+1.

Algebraic factorization used here (verified vs the reference):
    G = m_u*du - m_d*dd ,  H = m_r*dr - m_l*dl  (per xyz channel)
    normal = m_c * (H x G)
i.e. ONE cross product instead of four, and the 12 valid-map conv terms
reduce to shifted-mask multiplies.

Sharding: pure data parallel — one batch image per NeuronCore (8 cores).

Layout per core: partition p holds image rows [8p-1 .. 8p+8] (8 output rows
+ 1 halo row each side) so every stencil shift is a free-dim offset.
Columns are processed in chunks of CW with a 2-column halo (per-row pitch
P = CW + 4). The mask (f16, converted on host) stays SBUF-resident for the
whole image together with a one-element-left-shifted copy `ms`, so the
right/left mask views are plain aligned views.

Numerics: the diff/cross pipeline is kept in fp32 — the cross product
suffers catastrophic cancellation on near-parallel (H, G) pixels and f16
there produces O(0.1) absmax errors. The normalize uses
r = exp(-0.5*ln(s/256 + 1e-24) - ln(16)) = 1/sqrt(s + 2.56e-22) on the ACT
engine (squares pre-scaled by 1/16 so f16 partials cannot overflow).
"""

import os

import numpy as np

CH = 3
RPG = 8   # output rows per partition
NG = 10   # rows incl. halo
NCORES = 8

CW = int(os.environ.get("K_CW", "128"))

_CACHE = {}


def _emit(ctx, tc, pm, mk, out, H, W, cw):
    import concourse.bass as bass
    from concourse import mybir

    nc = tc.nc
    f32 = mybir.dt.float32
    f16 = mybir.dt.float16
    AF = mybir.ActivationFunctionType
    ALU = mybir.AluOpType

    NP = H // RPG          # partitions used (128 at full size)
    P = cw + 4             # per-row pitch in a column-chunk tile
    PM = W + 4             # per-row pitch of the resident mask tile
    nchunks = W // cw
    LN16 = float(np.log(16.0))

    def vw(t, pitch, r0, s0, nr=RPG, w=cw):
        return t.rearrange("p (r q) -> p r q", r=NG)[:, r0 : r0 + nr, s0 : s0 + w]

    zrow = {}  # dtype -> zeroed [NP, PM] scratch (for halo-row zeroing via DMA)

    def load_tile(pool, handle, base_off, dt, name, pitch, lo, ncols, soff):
        """Load rows [8p-1 .. 8p+8] x cols [lo .. lo+ncols) into slot soff."""
        t = pool.tile([NP, NG * pitch], dt, name=name, tag=name.split("_")[0])
        tv = t.rearrange("p (r q) -> p r q", r=NG)
        src = bass.AP(handle, base_off + (RPG - 1) * W + lo,
                      [[RPG * W, NP - 2], [W, NG], [1, ncols]])
        nc.sync.dma_start(out=tv[1 : NP - 1, :, soff : soff + ncols], in_=src)
        src0 = bass.AP(handle, base_off + lo, [[W * H, 1], [W, NG - 1], [1, ncols]])
        nc.sync.dma_start(out=tv[0:1, 1:NG, soff : soff + ncols], in_=src0)
        src1 = bass.AP(handle, base_off + (H - (NG - 1)) * W + lo,
                       [[W * H, 1], [W, NG - 1], [1, ncols]])
        nc.sync.dma_start(out=tv[NP - 1 : NP, 0 : NG - 1, soff : soff + ncols],
                          in_=src1)
        z = zrow[dt]
        nc.sync.dma_start(out=tv[0:1, 0:1, :], in_=z[0:1, 0:pitch])
        nc.sync.dma_start(out=tv[NP - 1 : NP, NG - 1 : NG, :], in_=z[0:1, 0:pitch])
        if soff > 0:
            nc.gpsimd.memset(tv[:, :, 0:soff], 0.0)
        if soff + ncols < pitch:
            nc.gpsimd.memset(tv[:, :, soff + ncols : pitch], 0.0)
        return t

    xin = ctx.enter_context(tc.tile_pool(name="xin", bufs=4))
    mres = ctx.enter_context(tc.tile_pool(name="mres", bufs=1))
    wpool = ctx.enter_context(tc.tile_pool(name="wpool", bufs=5))
    gh = ctx.enter_context(tc.tile_pool(name="gh", bufs=7))
    npool = ctx.enter_context(tc.tile_pool(name="npool", bufs=4))
    spool = ctx.enter_context(tc.tile_pool(name="spool", bufs=5))
    s32pool = ctx.enter_context(tc.tile_pool(name="s32pool", bufs=2))
    opool = ctx.enter_context(tc.tile_pool(name="opool", bufs=4))

    # per-partition bias constants for the ACT ops
    bias_eps = mres.tile([NP, 1], f32, name="bias_eps")
    nc.gpsimd.memset(bias_eps[:], 1e-24)
    bias_ln16 = mres.tile([NP, 1], f32, name="bias_ln16")
    nc.gpsimd.memset(bias_ln16[:], -LN16)

    for dt in (f32, f16):
        z = mres.tile([NP, PM], dt, name=f"zrow_{dt.name}")
        nc.gpsimd.memset(z[:], 0.0)
        zrow[dt] = z

    # resident mask (f16) + left-shifted copy: cols [-2 .. W+1] at slots 0..PM-1
    mt = load_tile(mres, mk, 0, f16, "mt", PM, 0, W, 2)
    ms = mres.tile([NP, NG * PM], f16, name="ms")
    nc.scalar.copy(ms[:, 0 : NG * PM - 1], mt[:, 1 : NG * PM])
    nc.gpsimd.memset(ms[:, NG * PM - 1 : NG * PM], 0.0)

    for k in range(nchunks):
        j0 = k * cw
        lo = max(j0 - 2, 0)
        hi = min(j0 + cw + 1, W - 1)
        ncols = hi - lo + 1
        soff = lo - (j0 - 2)

        xts = [load_tile(xin, pm, c * H * W, f32, f"x_{k}_{c}", P, lo, ncols, soff)
               for c in range(CH)]

        # mask views for this chunk (slot = col + 2 in the resident tiles)
        mU = vw(mt, PM, 0, j0 + 2)
        mD = vw(mt, PM, 2, j0 + 2)
        mC = vw(mt, PM, 1, j0 + 2)
        mR = vw(ms, PM, 1, j0 + 2)   # = m at col j0+1
        mL = vw(ms, PM, 1, j0)       # = m at col j0-1

        Gs, Hs = [], []
        for c in range(CH):
            xt = xts[c]
            xC = vw(xt, P, 1, 2)
            xU = vw(xt, P, 0, 2)
            xD = vw(xt, P, 2, 2)
            xR = vw(xt, P, 1, 3)
            xL = vw(xt, P, 1, 1)

            w3 = lambda t: t.rearrange("p (r q) -> p r q", r=RPG)

            def wt(nm):
                return wpool.tile([NP, RPG * cw], f32, name=f"{nm}_{k}_{c}", tag="w")

            du = wt("du"); nc.vector.tensor_sub(w3(du), xU, xC)
            dd = wt("dd"); nc.vector.tensor_sub(w3(dd), xD, xC)
            t1 = wt("t1"); nc.vector.tensor_tensor(w3(t1), mU, w3(du), ALU.mult)
            t2 = wt("t2"); nc.vector.tensor_tensor(w3(t2), mD, w3(dd), ALU.mult)
            G = gh.tile([NP, RPG * cw], f32, name=f"G_{k}_{c}", tag="gh")
            nc.vector.tensor_sub(G[:], t1[:], t2[:])

            dr = wt("dr"); nc.vector.tensor_sub(w3(dr), xR, xC)
            dl = wt("dl"); nc.vector.tensor_sub(w3(dl), xL, xC)
            t3 = wt("t3"); nc.vector.tensor_tensor(w3(t3), mR, w3(dr), ALU.mult)
            t4 = wt("t4"); nc.vector.tensor_tensor(w3(t4), mL, w3(dl), ALU.mult)
            Ht = gh.tile([NP, RPG * cw], f32, name=f"H_{k}_{c}", tag="gh")
            nc.vector.tensor_sub(Ht[:], t3[:], t4[:])
            Gs.append(G)
            Hs.append(Ht)

        # n = H x G
        ns = []
        for c in range(CH):
            a, b = (c + 1) % 3, (c + 2) % 3
            ta = wpool.tile([NP, RPG * cw], f32, name=f"ca_{k}_{c}", tag="w")
            nc.vector.tensor_tensor(ta[:], Hs[a][:], Gs[b][:], ALU.mult)
            tb = wpool.tile([NP, RPG * cw], f32, name=f"cb_{k}_{c}", tag="w")
            nc.vector.tensor_tensor(tb[:], Hs[b][:], Gs[a][:], ALU.mult)
            n_c = npool.tile([NP, RPG * cw], f32, name=f"n_{k}_{c}", tag="n")
            nc.vector.tensor_sub(n_c[:], ta[:], tb[:])
            ns.append(n_c)

        # r = 1/sqrt(s/256 + 1e-24)/16 = 1/sqrt(s + 2.56e-22)
        sq = []
        for c in range(CH):
            s_c = spool.tile([NP, RPG * cw], f32, name=f"sq_{k}_{c}", tag="s")
            nc.scalar.activation(s_c[:], ns[c][:], AF.Square, scale=0.0625)
            sq.append(s_c)
        s01 = spool.tile([NP, RPG * cw], f32, name=f"s01_{k}", tag="s")
        nc.vector.tensor_add(s01[:], sq[0][:], sq[1][:])
        s2 = spool.tile([NP, RPG * cw], f32, name=f"s2_{k}", tag="s")
        nc.vector.tensor_add(s2[:], s01[:], sq[2][:])
        lns = s32pool.tile([NP, RPG * cw], f32, name=f"lns_{k}", tag="s32")
        nc.scalar.activation(lns[:], s2[:], AF.Ln, bias=bias_eps[:])
        r = s32pool.tile([NP, RPG * cw], f32, name=f"r_{k}", tag="s32")
        nc.scalar.activation(r[:], lns[:], AF.Exp, scale=-0.5, bias=bias_ln16[:])
        rm = s32pool.tile([NP, RPG * cw], f32, name=f"rm_{k}", tag="s32")
        nc.vector.tensor_tensor(rm.rearrange("p (r q) -> p r q", r=RPG), mC,
                                r.rearrange("p (r q) -> p r q", r=RPG), ALU.mult)

        for c in range(CH):
            o = opool.tile([NP, RPG * cw], f32, name=f"o_{k}_{c}", tag="o")
            nc.vector.tensor_tensor(o[:], ns[c][:], rm[:], ALU.mult)
            dst = bass.AP(out, c * H * W + j0, [[RPG * W, NP], [W, RPG], [1, cw]])
            nc.sync.dma_start(out=dst, in_=o.rearrange("p (r q) -> p r q", r=RPG))


def build(H=1024, W=1024, cw=None):
    cw = cw or CW
    key = (H, W, cw)
    if key in _CACHE:
        return _CACHE[key]
    from contextlib import ExitStack

    import concourse.tile as tile
    from concourse import bacc, mybir

    nc = bacc.Bacc("TRN2", target_bir_lowering=False, debug=False,
                   num_devices=NCORES)
    pm = nc.dram_tensor("posmap", [CH, H, W], mybir.dt.float32,
                        kind="ExternalInput")
    mk = nc.dram_tensor("mask", [H, W], mybir.dt.float16, kind="ExternalInput")
    out = nc.dram_tensor("out", [CH, H, W], mybir.dt.float32,
                         kind="ExternalOutput")
    with tile.TileContext(nc) as tc:
        with ExitStack() as ctx:
            _emit(ctx, tc, pm, mk, out, H, W, cw)
    nc.compile()
    _CACHE[key] = nc
    return nc


def kernel(posmap: np.ndarray, mask: np.ndarray, _trace: bool = False):
    nc = build(posmap.shape[2], posmap.shape[3])
    from concourse.bass_utils import run_bass_kernel_spmd

    mask_f16 = np.ascontiguousarray(mask.astype(np.float16))
    nb = posmap.shape[0]
    in_maps = [
        {"posmap": np.ascontiguousarray(posmap[b]), "mask": mask_f16}
        for b in range(nb)
    ]
    try:
        res = run_bass_kernel_spmd(nc, in_maps, core_ids=list(range(nb)),
                                   trace=_trace)
    except ModuleNotFoundError:
        res = run_bass_kernel_spmd(nc, in_maps, core_ids=list(range(nb)),
                                   trace=False)
    out = np.stack([res.results[b]["out"] for b in range(nb)], axis=0)
    if _trace:
        kernel.last_exec_time_ns = res.exec_time_ns
        kernel.last_trace = res.instructions_and_trace
    return out


# revision 19
# speedup vs baseline: 11378.6112x; 11378.6112x over previous
"""Trainium2 Bass kernel for nn_MaskedPosmap2Normal.

Per batch image b and pixel (i,j), the reference computes
    d_k = neighbor_k - center  (k = right, up, left, down; zero-padded)
    normal = sum_k valid_k * (d_k x d_{k+1 mod 4})
    out = normal / max(||normal||, 1e-12)
where valid_k is the AND of the 3 mask bits bracketing directions k, k+1.

Algebraic factorization used here (verified vs the reference):
    G = m_u*du - m_d*dd ,  H = m_r*dr - m_l*dl  (per xyz channel)
    normal = m_c * (H x G)
i.e. ONE cross product instead of four, and the 12 valid-map conv terms
reduce to shifted-mask multiplies.

Sharding: pure data parallel — one batch image per NeuronCore (8 cores).

Layout per core: partition p holds image rows [8p-1 .. 8p+8] (8 output rows
+ 1 halo row each side) so every stencil shift is a free-dim offset.
Columns are processed in chunks of CW with a 2-column halo (per-row pitch
P = CW + 4). The mask (f16, converted on host) stays SBUF-resident for the
whole image together with a one-element-left-shifted copy `ms`, so the
right/left mask views are plain aligned views.

Numerics: the diff/cross pipeline is kept in fp32 — the cross product
suffers catastrophic cancellation on near-parallel (H, G) pixels and f16
there produces O(0.1) absmax errors. The normalize uses
r = exp(-0.5*ln(s/256 + 1e-24) - ln(16)) = 1/sqrt(s + 2.56e-22) on the ACT
engine (squares pre-scaled by 1/16 so f16 partials cannot overflow).
"""

import os

import numpy as np

CH = 3
RPG = 8   # output rows per partition
NG = 10   # rows incl. halo
NCORES = 8

CW = int(os.environ.get("K_CW", "128"))
# comma-separated op-sites to run on GPSIMD: subset of {d,t,x,s,o}
GP_SITES = frozenset(x for x in os.environ.get("K_GP", "").split(",") if x)

_CACHE = {}


def _emit(ctx, tc, pm, mk, out, H, W, cw, reps=1):
    import concourse.bass as bass
    from concourse import mybir

    nc = tc.nc
    f32 = mybir.dt.float32
    f16 = mybir.dt.float16
    AF = mybir.ActivationFunctionType
    ALU = mybir.AluOpType

    def eng(site):
        return nc.gpsimd if site in GP_SITES else nc.vector

    NP = H // RPG          # partitions used (128 at full size)
    P = cw + 4             # per-row pitch in a column-chunk tile
    PM = W + 4             # per-row pitch of the resident mask tile
    nchunks = W // cw
    LN16 = float(np.log(16.0))

    def vw(t, pitch, r0, s0, nr=RPG, w=cw):
        return t.rearrange("p (r q) -> p r q", r=NG)[:, r0 : r0 + nr, s0 : s0 + w]

    zrow = {}  # dtype -> zeroed [NP, PM] scratch (for halo-row zeroing via DMA)

    def load_tile(pool, handle, base_off, dt, name, pitch, lo, ncols, soff):
        """Load rows [8p-1 .. 8p+8] x cols [lo .. lo+ncols) into slot soff."""
        t = pool.tile([NP, NG * pitch], dt, name=name, tag=name.split("_")[0])
        tv = t.rearrange("p (r q) -> p r q", r=NG)
        src = bass.AP(handle, base_off + (RPG - 1) * W + lo,
                      [[RPG * W, NP - 2], [W, NG], [1, ncols]])
        nc.sync.dma_start(out=tv[1 : NP - 1, :, soff : soff + ncols], in_=src)
        src0 = bass.AP(handle, base_off + lo, [[W * H, 1], [W, NG - 1], [1, ncols]])
        nc.sync.dma_start(out=tv[0:1, 1:NG, soff : soff + ncols], in_=src0)
        src1 = bass.AP(handle, base_off + (H - (NG - 1)) * W + lo,
                       [[W * H, 1], [W, NG - 1], [1, ncols]])
        nc.sync.dma_start(out=tv[NP - 1 : NP, 0 : NG - 1, soff : soff + ncols],
                          in_=src1)
        z = zrow[dt]
        nc.sync.dma_start(out=tv[0:1, 0:1, :], in_=z[0:1, 0:pitch])
        nc.sync.dma_start(out=tv[NP - 1 : NP, NG - 1 : NG, :], in_=z[0:1, 0:pitch])
        if soff > 0:
            nc.gpsimd.memset(tv[:, :, 0:soff], 0.0)
        if soff + ncols < pitch:
            nc.gpsimd.memset(tv[:, :, soff + ncols : pitch], 0.0)
        return t

    xin = ctx.enter_context(tc.tile_pool(name="xin", bufs=4))
    mres = ctx.enter_context(tc.tile_pool(name="mres", bufs=1))
    wpool = ctx.enter_context(tc.tile_pool(name="wpool", bufs=5))
    gh = ctx.enter_context(tc.tile_pool(name="gh", bufs=7))
    npool = ctx.enter_context(tc.tile_pool(name="npool", bufs=4))
    spool = ctx.enter_context(tc.tile_pool(name="spool", bufs=5))
    s32pool = ctx.enter_context(tc.tile_pool(name="s32pool", bufs=2))
    opool = ctx.enter_context(tc.tile_pool(name="opool", bufs=4))

    # per-partition bias constants for the ACT ops
    bias_eps = mres.tile([NP, 1], f32, name="bias_eps")
    nc.gpsimd.memset(bias_eps[:], 1e-24)
    bias_ln16 = mres.tile([NP, 1], f32, name="bias_ln16")
    nc.gpsimd.memset(bias_ln16[:], -LN16)

    for dt in (f32, f16):
        z = mres.tile([NP, PM], dt, name=f"zrow_{dt.name}")
        nc.gpsimd.memset(z[:], 0.0)
        zrow[dt] = z

    # resident mask (f16) + left-shifted copy: cols [-2 .. W+1] at slots 0..PM-1
    mt = load_tile(mres, mk, 0, f16, "mt", PM, 0, W, 2)
    ms = mres.tile([NP, NG * PM], f16, name="ms")
    nc.scalar.copy(ms[:, 0 : NG * PM - 1], mt[:, 1 : NG * PM])
    nc.gpsimd.memset(ms[:, NG * PM - 1 : NG * PM], 0.0)

    for rep in range(reps):
      for k0 in range(nchunks):
        k = rep * nchunks + k0
        j0 = k0 * cw
        lo = max(j0 - 2, 0)
        hi = min(j0 + cw + 1, W - 1)
        ncols = hi - lo + 1
        soff = lo - (j0 - 2)

        xts = [load_tile(xin, pm, c * H * W, f32, f"x_{k}_{c}", P, lo, ncols, soff)
               for c in range(CH)]

        # mask views for this chunk (slot = col + 2 in the resident tiles)
        mU = vw(mt, PM, 0, j0 + 2)
        mD = vw(mt, PM, 2, j0 + 2)
        mC = vw(mt, PM, 1, j0 + 2)
        mR = vw(ms, PM, 1, j0 + 2)   # = m at col j0+1
        mL = vw(ms, PM, 1, j0)       # = m at col j0-1

        Gs, Hs = [], []
        for c in range(CH):
            xt = xts[c]
            xC = vw(xt, P, 1, 2)
            xU = vw(xt, P, 0, 2)
            xD = vw(xt, P, 2, 2)
            xR = vw(xt, P, 1, 3)
            xL = vw(xt, P, 1, 1)

            w3 = lambda t: t.rearrange("p (r q) -> p r q", r=RPG)

            def wt(nm):
                return wpool.tile([NP, RPG * cw], f32, name=f"{nm}_{k}_{c}", tag="w")

            du = wt("du"); eng("d").tensor_sub(w3(du), xU, xC)
            dd = wt("dd"); eng("d").tensor_sub(w3(dd), xD, xC)
            t1 = wt("t1"); eng("t").tensor_tensor(w3(t1), mU, w3(du), ALU.mult)
            t2 = wt("t2"); eng("t").tensor_tensor(w3(t2), mD, w3(dd), ALU.mult)
            G = gh.tile([NP, RPG * cw], f32, name=f"G_{k}_{c}", tag="gh")
            eng("g").tensor_sub(G[:], t1[:], t2[:])

            dr = wt("dr"); eng("d").tensor_sub(w3(dr), xR, xC)
            dl = wt("dl"); eng("d").tensor_sub(w3(dl), xL, xC)
            t3 = wt("t3"); eng("t").tensor_tensor(w3(t3), mR, w3(dr), ALU.mult)
            t4 = wt("t4"); eng("t").tensor_tensor(w3(t4), mL, w3(dl), ALU.mult)
            Ht = gh.tile([NP, RPG * cw], f32, name=f"H_{k}_{c}", tag="gh")
            eng("g").tensor_sub(Ht[:], t3[:], t4[:])
            Gs.append(G)
            Hs.append(Ht)

        # n = H x G
        ns = []
        for c in range(CH):
            a, b = (c + 1) % 3, (c + 2) % 3
            ta = wpool.tile([NP, RPG * cw], f32, name=f"ca_{k}_{c}", tag="w")
            eng("x").tensor_tensor(ta[:], Hs[a][:], Gs[b][:], ALU.mult)
            tb = wpool.tile([NP, RPG * cw], f32, name=f"cb_{k}_{c}", tag="w")
            eng("x").tensor_tensor(tb[:], Hs[b][:], Gs[a][:], ALU.mult)
            n_c = npool.tile([NP, RPG * cw], f32, name=f"n_{k}_{c}", tag="n")
            eng("n").tensor_sub(n_c[:], ta[:], tb[:])
            ns.append(n_c)

        # r = 1/sqrt(s/256 + 1e-24)/16 = 1/sqrt(s + 2.56e-22)
        sq = []
        for c in range(CH):
            s_c = spool.tile([NP, RPG * cw], f32, name=f"sq_{k}_{c}", tag="s")
            nc.scalar.activation(s_c[:], ns[c][:], AF.Square, scale=0.0625)
            sq.append(s_c)
        s01 = spool.tile([NP, RPG * cw], f32, name=f"s01_{k}", tag="s")
        eng("s").tensor_add(s01[:], sq[0][:], sq[1][:])
        s2 = spool.tile([NP, RPG * cw], f32, name=f"s2_{k}", tag="s")
        eng("s").tensor_add(s2[:], s01[:], sq[2][:])
        lns = s32pool.tile([NP, RPG * cw], f32, name=f"lns_{k}", tag="s32")
        nc.scalar.activation(lns[:], s2[:], AF.Ln, bias=bias_eps[:])
        r = s32pool.tile([NP, RPG * cw], f32, name=f"r_{k}", tag="s32")
        nc.scalar.activation(r[:], lns[:], AF.Exp, scale=-0.5, bias=bias_ln16[:])
        rm = s32pool.tile([NP, RPG * cw], f32, name=f"rm_{k}", tag="s32")
        eng("s").tensor_tensor(rm.rearrange("p (r q) -> p r q", r=RPG), mC,
                               r.rearrange("p (r q) -> p r q", r=RPG), ALU.mult)

        for c in range(CH):
            o = opool.tile([NP, RPG * cw], f32, name=f"o_{k}_{c}", tag="o")
            eng("o").tensor_tensor(o[:], ns[c][:], rm[:], ALU.mult)
            dst = bass.AP(out, c * H * W + j0, [[RPG * W, NP], [W, RPG], [1, cw]])
            nc.sync.dma_start(out=dst, in_=o.rearrange("p (r q) -> p r q", r=RPG))


def build(H=1024, W=1024, cw=None, reps=1):
    cw = cw or CW
    key = (H, W, cw, reps)
    if key in _CACHE:
        return _CACHE[key]
    from contextlib import ExitStack

    import concourse.tile as tile
    from concourse import bacc, mybir

    nc = bacc.Bacc("TRN2", target_bir_lowering=False, debug=False,
                   num_devices=NCORES)
    pm = nc.dram_tensor("posmap", [CH, H, W], mybir.dt.float32,
                        kind="ExternalInput")
    mk = nc.dram_tensor("mask", [H, W], mybir.dt.float16, kind="ExternalInput")
    out = nc.dram_tensor("out", [CH, H, W], mybir.dt.float32,
                         kind="ExternalOutput")
    with tile.TileContext(nc) as tc:
        with ExitStack() as ctx:
            _emit(ctx, tc, pm, mk, out, H, W, cw, reps)
    nc.compile()
    _CACHE[key] = nc
    return nc


def kernel(posmap: np.ndarray, mask: np.ndarray, _trace: bool = False):
    nc = build(posmap.shape[2], posmap.shape[3])
    from concourse.bass_utils import run_bass_kernel_spmd

    mask_f16 = np.ascontiguousarray(mask.astype(np.float16))
    nb = posmap.shape[0]
    in_maps = [
        {"posmap": np.ascontiguousarray(posmap[b]), "mask": mask_f16}
        for b in range(nb)
    ]
    try:
        res = run_bass_kernel_spmd(nc, in_maps, core_ids=list(range(nb)),
                                   trace=_trace)
    except ModuleNotFoundError:
        res = run_bass_kernel_spmd(nc, in_maps, core_ids=list(range(nb)),
                                   trace=False)
    out = np.stack([res.results[b]["out"] for b in range(nb)], axis=0)
    if _trace:
        kernel.last_exec_time_ns = res.exec_time_ns
        kernel.last_trace = res.instructions_and_trace
    return out
